# revision 1
# baseline (speedup 1.0000x reference)
"""BYOL-style cosine MSE loss on 8 Trainium2 NeuronCores.

Full inputs: online_output [16384, 1024] f32, target_output [16384, 1024] f32.
Output: scalar f32 = mean(2 - 2*cos_row(online, target)) / 0.05.

Sharding: data-parallel along N. Each of the 8 cores gets 2048 rows and
computes, per row r: dot_r = sum_d o*t, n1sq_r = sum_d o*o, n2sq_r = sum_d t*t
via fused multiply-reduce ops (DVE scalar_tensor_tensor with accum_out, ACT
Square with accum_out), overlapped with 1 MiB HWDGE DMA loads under a Tile
pipeline. The per-row stats ([2, 128, 24] per core) come back to the host,
which finishes the cosine + scalar mean in float64 (the "all-reduce" of the
sharding hint is a trivial 8-way host reduction of 24 KB/core).
"""

import numpy as np

P = 128          # SBUF partitions
D = 1024         # feature dim
N = 16384        # total rows
N_CORES = 8
N_LOC = N // N_CORES          # 2048 rows per core
N_TILES = N_LOC // P          # 16 row-tiles per core

TEMP = 0.05
EPS = 1e-8

_NC_CACHE = {}


def _legalize_waits(nc, max_waits=1):
    """Split multi-wait instructions into single-wait NOPs + the instruction.

    This container's walrus build accepts at most one semaphore wait per
    instruction, while Tile emits instructions waiting on several producer
    sems. AND-of-waits is preserved by stalling the same engine's sequencer
    on a chain of single-wait NOPs immediately before the instruction.
    """
    import concourse.mybir as mybir

    ctr = 0
    for f in nc.m.functions:
        for b in f.blocks:
            ins_list = b.instructions
            i = 0
            while i < len(ins_list):
                inst = ins_list[i]
                si = inst.sync_info
                if (
                    si is not None
                    and si.on_wait is not None
                    and len(si.on_wait) > max_waits
                ):
                    waits = si.on_wait
                    extra = [waits.pop() for _ in range(len(waits) - max_waits)]
                    for w in reversed(extra):
                        ctr += 1
                        noop = mybir.InstNoOp(
                            name=f"waitsplit_{ctr}",
                            engine=inst.engine,
                            ins=[],
                            outs=[],
                            sync_info=mybir.SyncInfo(on_wait=[w], on_update=[]),
                        )
                        ins_list.insert(i, noop)
                        i += 1
                i += 1


def _trim_tail_barrier(nc):
    """Shrink the TileContext exit sequence to just the SP DMA-drain.

    Tile emits: drain -> all-engine barrier -> sem clears (Pool ISA) ->
    all-engine barrier. Everything after the drain exists to leave the
    semaphores cleared for the NEXT execution; instead, relocate the clear
    to the kernel START (on Pool, before the existing start barrier, so
    every engine's first sem use still happens after the clear) and delete
    both exit barriers. The SP drain must stay: it waits for the DMA queues,
    guaranteeing the stats write landed before the NEFF completes. Saves
    ~3 us of measured tail per execution.
    """
    import concourse.mybir as mybir

    moved = []
    for f in nc.m.functions:
        end_blocks = [b for b in f.blocks if b.name.endswith("_end")]
        main_blocks = [b for b in f.blocks if b.name == "main"]
        if not end_blocks or not main_blocks:
            continue
        ins_list = end_blocks[0].instructions
        # grab the Pool sem-clear/dma-reset ISA instructions for relocation
        moved = [
            ins
            for ins in ins_list
            if isinstance(ins, mybir.InstISA)
            and ins.engine == mybir.EngineType.Pool
        ]
        # truncate right after the first SP drain (the DMA-queue quiesce)
        for i, ins in enumerate(ins_list):
            if isinstance(ins, mybir.InstDrain) and ins.engine == mybir.EngineType.SP:
                del ins_list[i + 1 :]
                if _DROP_DRAIN["v"]:
                    # Drop the receipt wait too: the stats write lands ~2 us
                    # after issue, while the walrus postamble (barrier + 253
                    # sem clears, >=6 us) must still run before the NEFF can
                    # complete - the data is down long before the host can
                    # see "done". Postamble then starts at stats-issue time.
                    del ins_list[i]
                break
        # insert the clears into main before Pool's start-barrier gather
        main_ins = main_blocks[0].instructions
        for i, ins in enumerate(main_ins):
            if (
                isinstance(ins, mybir.InstEventSemaphore)
                and ins.engine == mybir.EngineType.Pool
            ):
                for k, m in enumerate(moved):
                    main_ins.insert(i + k, m)
                break
    return nc


_MAX_SEM = {"n": 256}
_DROP_DRAIN = {"v": False}


def _compact_sems(nc, keep_below=3, base=3):
    """Densely remap semaphore ids to start at `base`.

    Bass allocates sem ids from a pool starting around 150, but the walrus
    NEFF postamble zeroes every semaphore below --max-sem-num with one
    EventSemaphore instruction each, split across engines (~115 ns/sem on
    the critical engine). Compacting our ~14 sems to ids 3..16 and capping
    --max-sem-num shrinks that postamble from ~6 us to sub-us.
    """
    mapping = {}
    for f in nc.m.functions:
        for b in f.blocks:
            for ins in b.instructions:
                si = ins.sync_info
                if not si:
                    continue
                for lst in (si.on_wait, si.on_update):
                    if not lst:
                        continue
                    for e in lst:
                        i = getattr(e, "id", None)
                        if i is None or i < keep_below:
                            continue
                        if i not in mapping:
                            mapping[i] = base + len(mapping)
                        e.id = mapping[i]
    return (max(mapping.values()) + 1) if mapping else base


def _slim_exit_drain(nc):
    """Keep only the stats-DMA completion waits on the exit drain.

    Tile's exit drain waits on every sem lane the kernel touched (10 waits
    -> a ~1.2 us serial NOP chain on Sync after legalization). All of them
    except the output DMAs' completion lanes are transitively implied: the
    stats DMAs' own waits required all compute, which required all input
    loads. Dropping the redundant waits lets Sync reach the NEFF postamble
    barrier ~1 us earlier.
    """
    import concourse.mybir as mybir

    for f in nc.m.functions:
        end_blocks = [b for b in f.blocks if b.name.endswith("_end")]
        body_blocks = [
            b for b in f.blocks if not b.name.endswith("_end") and b.name != "main"
        ]
        if not end_blocks:
            continue
        # completion sem lanes of the DMAs that write the "stats" output
        stats_lanes = set()
        for b in body_blocks:
            for ins in b.instructions:
                if not isinstance(ins, mybir.InstDMACopy):
                    continue
                outs = getattr(ins, "outs", [])
                if not any("stats" in str(getattr(o, "memref", "")) for o in outs):
                    continue
                si = ins.sync_info
                if si and si.on_update:
                    for u in si.on_update:
                        stats_lanes.add(u.id)
        if not stats_lanes:
            continue
        for b in end_blocks:
            for ins in b.instructions:
                if (
                    isinstance(ins, mybir.InstDrain)
                    and ins.engine == mybir.EngineType.SP
                ):
                    si = ins.sync_info
                    if si and si.on_wait:
                        kept = [w for w in si.on_wait if w.id in stats_lanes]
                        if kept:
                            while len(si.on_wait) > 0:
                                si.on_wait.pop()
                            for w in kept:
                                si.on_wait.append(w)
                    break
    return nc


def _build_nc(legalize=True, io_bufs=4, tail_singles=2, balance=True, trim_tail=True):
    import concourse.bass as bass
    import concourse.mybir as mybir
    from concourse.tile import TileContext

    fp32 = mybir.dt.float32
    # chunk schedule: 1 MiB (2-tile) loads, with the last tiles loaded singly
    # so the post-DMA compute tail is short
    chunks = [2] * ((N_TILES - tail_singles) // 2) + [1] * tail_singles
    assert sum(chunks) == N_TILES
    per_half = N_TILES // 2
    nc = bass.Bass(enable_partition_id=False)
    o_in = nc.declare_dram_parameter("online", [N_LOC, D], fp32, isOutput=False)
    t_in = nc.declare_dram_parameter("target", [N_LOC, D], fp32, isOutput=False)
    # stats[h]: contiguous [P, 24] per half h; cols [0:8] dot, [8:16] sum o^2,
    # [16:24] sum t^2, for tiles h*8..h*8+7
    # one extra column per half holds the ACT-computed second half of the
    # last tile's sum t^2 (host adds it back)
    stats = nc.declare_dram_parameter(
        "stats", [2, P, 3 * per_half + 1], fp32, isOutput=True
    )

    o_all = o_in.rearrange("(t p) d -> p t d", p=P)
    t_all = t_in.rearrange("(t p) d -> p t d", p=P)

    with TileContext(nc) as tc:
        with (
            tc.tile_pool(name="io", bufs=io_bufs) as io_pool,
            tc.tile_pool(name="scr", bufs=2) as scr_pool,
            tc.tile_pool(name="acc", bufs=1) as acc_pool,
        ):
            # Two accumulators so the first half's stats DMA out can overlap
            # the second half's compute.
            accs = [
                acc_pool.tile(
                    [P, 3 * per_half + 1], fp32, name=f"acc{h}", tag=f"acc{h}"
                )
                for h in range(2)
            ]
            t0 = 0
            for cg in chunks:
                o_tile = io_pool.tile([P, 2 * D], fp32, name="o_tile")
                t_tile = io_pool.tile([P, 2 * D], fp32, name="t_tile")
                nc.sync.dma_start(
                    out=o_tile[:, 0 : cg * D].rearrange("p (t d) -> p t d", t=cg),
                    in_=o_all[:, t0 : t0 + cg],
                )
                nc.sync.dma_start(
                    out=t_tile[:, 0 : cg * D].rearrange("p (t d) -> p t d", t=cg),
                    in_=t_all[:, t0 : t0 + cg],
                )
                for gi in range(cg):
                    idx = t0 + gi
                    h = idx // per_half
                    col = idx % per_half
                    acc = accs[h]
                    osl = o_tile[:, gi * D : (gi + 1) * D]
                    tsl = t_tile[:, gi * D : (gi + 1) * D]
                    prod = scr_pool.tile([P, D], fp32, name="prod")
                    sq_o = scr_pool.tile([P, D], fp32, name="sq_o")
                    sq_t = scr_pool.tile([P, D], fp32, name="sq_t")
                    nc.vector.scalar_tensor_tensor(
                        out=prod[:],
                        in0=osl,
                        scalar=1.0,
                        in1=tsl,
                        op0=mybir.AluOpType.mult,
                        op1=mybir.AluOpType.mult,
                        accum_out=acc[:, col : col + 1],
                    )
                    nc.scalar.activation(
                        sq_o[:],
                        osl,
                        mybir.ActivationFunctionType.Square,
                        accum_out=acc[:, per_half + col : per_half + col + 1],
                    )
                    # Balance the two squares across ACT and DVE: ACT alone
                    # otherwise outlives the DMA stream; alternating t^2 puts
                    # ~27us on each engine.
                    if balance and idx % 2 == 1 and col == per_half - 1:
                        # last tile of the half: split t^2 across DVE and ACT
                        # so neither engine carries a 2-op chain after the
                        # final load
                        hw = D // 2
                        nc.vector.scalar_tensor_tensor(
                            out=sq_t[:, :hw],
                            in0=tsl[:, :hw],
                            scalar=1.0,
                            in1=tsl[:, :hw],
                            op0=mybir.AluOpType.mult,
                            op1=mybir.AluOpType.mult,
                            accum_out=acc[
                                :, 2 * per_half + col : 2 * per_half + col + 1
                            ],
                        )
                        nc.scalar.activation(
                            sq_t[:, hw:],
                            tsl[:, hw:],
                            mybir.ActivationFunctionType.Square,
                            accum_out=acc[:, 3 * per_half : 3 * per_half + 1],
                        )
                    elif balance and idx % 2 == 1:
                        nc.vector.scalar_tensor_tensor(
                            out=sq_t[:],
                            in0=tsl,
                            scalar=1.0,
                            in1=tsl,
                            op0=mybir.AluOpType.mult,
                            op1=mybir.AluOpType.mult,
                            accum_out=acc[
                                :, 2 * per_half + col : 2 * per_half + col + 1
                            ],
                        )
                    else:
                        nc.scalar.activation(
                            sq_t[:],
                            tsl,
                            mybir.ActivationFunctionType.Square,
                            accum_out=acc[
                                :, 2 * per_half + col : 2 * per_half + col + 1
                            ],
                        )
                t0 += cg
            # Emit both stats DMAs after every load: their compute-waits
            # then stall the SP sequencer only once it has nothing left to
            # issue (emitting stats[0] mid-loop blocked the remaining load
            # issues behind its waits — a ~1.2 us mid-stream DMA gap).
            # stats[0] still executes as soon as tiles 0..7 finish,
            # overlapping the second half's compute.
            nc.sync.dma_start(out=stats[0], in_=accs[0][:])
            nc.sync.dma_start(out=stats[1], in_=accs[1][:])
    if trim_tail:
        _trim_tail_barrier(nc)
        _slim_exit_drain(nc)
    if legalize:
        _legalize_waits(nc)
    _MAX_SEM["n"] = _compact_sems(nc) + 8  # headroom for walrus-internal sems
    return nc


def _build_nc_raw(legalize=True, g=2, bufs=4):
    """Raw-Bass (no TileContext) variant: manual semaphores, no all-engine
    barrier preamble/tail. Same math and I/O contract as _build_nc."""
    import concourse.bass as bass
    import concourse.mybir as mybir

    fp32 = mybir.dt.float32
    n_chunks = N_TILES // g
    # detect_race_conditions=False: the only "races" here are same-engine
    # WAW on the prod/sq scratch tiles, which hardware executes in order;
    # the detector has no same-engine-order model and rejects them.
    nc = bass.Bass(enable_partition_id=False, detect_race_conditions=False)
    o_in = nc.declare_dram_parameter("online", [N_LOC, D], fp32, isOutput=False)
    t_in = nc.declare_dram_parameter("target", [N_LOC, D], fp32, isOutput=False)
    stats = nc.declare_dram_parameter("stats", [P, 3 * N_TILES], fp32, isOutput=True)

    o_view = o_in.rearrange("(c g p) d -> c p g d", g=g, p=P)
    t_view = t_in.rearrange("(c g p) d -> c p g d", g=g, p=P)

    with (
        nc.sbuf_tensor([P, bufs * g * D], fp32) as o_buf,
        nc.sbuf_tensor([P, bufs * g * D], fp32) as t_buf,
        nc.sbuf_tensor([P, D], fp32) as prod,
        nc.sbuf_tensor([P, D], fp32) as sq,
        nc.sbuf_tensor([P, 3 * N_TILES], fp32) as acc,
        nc.semaphore() as dve_sem,
        nc.semaphore() as act_sem,
        nc.Block() as block,
    ):
        # One DMA sem per buffer slot: a slot's threshold must only count
        # that slot's own loads (SDMA engines skew across queued DMAs, so a
        # single shared sem can hit a chunk's threshold with increments
        # from later chunks' transfers).
        dma_sems = [nc.alloc_semaphore(f"dma_s{i}") for i in range(bufs)]

        def slot_thresh(c):
            return 32 * (c // bufs + 1)

        def o_slot(s, gi):
            return o_buf[:, (s * g + gi) * D : (s * g + gi + 1) * D]

        def t_slot(s, gi):
            return t_buf[:, (s * g + gi) * D : (s * g + gi + 1) * D]

        @block.gpsimd
        def _(gpsimd):
            for c in range(n_chunks):
                if c >= bufs:
                    gpsimd.wait_ge(dve_sem, c - bufs + 1)
                    gpsimd.wait_ge(act_sem, c - bufs + 1)
                s = c % bufs
                o_dst = o_buf[:, s * g * D : (s + 1) * g * D].rearrange(
                    "p (g d) -> p g d", g=g
                )
                t_dst = t_buf[:, s * g * D : (s + 1) * g * D].rearrange(
                    "p (g d) -> p g d", g=g
                )
                gpsimd.dma_start(out=o_dst, in_=o_view[c]).then_inc(dma_sems[s], 16)
                gpsimd.dma_start(out=t_dst, in_=t_view[c]).then_inc(dma_sems[s], 16)
            gpsimd.wait_ge(dve_sem, n_chunks)
            gpsimd.wait_ge(act_sem, n_chunks)
            final = 32 * (n_chunks // bufs) + 16
            gpsimd.dma_start(out=stats[:], in_=acc[:]).then_inc(dma_sems[0], 16)
            gpsimd.wait_ge(dma_sems[0], final)

        @block.vector
        def _(vector):
            for c in range(n_chunks):
                s = c % bufs
                vector.wait_ge(dma_sems[s], slot_thresh(c))
                for gi in range(g):
                    idx = c * g + gi
                    ins = nc.vector.scalar_tensor_tensor(
                        out=prod[:],
                        in0=o_slot(s, gi),
                        scalar=1.0,
                        in1=t_slot(s, gi),
                        op0=mybir.AluOpType.mult,
                        op1=mybir.AluOpType.mult,
                        accum_out=acc[:, idx : idx + 1],
                    )
                    if gi == g - 1:
                        ins.then_inc(dve_sem, 1)

        @block.scalar
        def _(scalar):
            for c in range(n_chunks):
                s = c % bufs
                scalar.wait_ge(dma_sems[s], slot_thresh(c))
                for gi in range(g):
                    idx = c * g + gi
                    nc.scalar.activation(
                        sq[:],
                        o_slot(s, gi),
                        mybir.ActivationFunctionType.Square,
                        accum_out=acc[:, N_TILES + idx : N_TILES + idx + 1],
                    )
                    ins = nc.scalar.activation(
                        sq[:],
                        t_slot(s, gi),
                        mybir.ActivationFunctionType.Square,
                        accum_out=acc[:, 2 * N_TILES + idx : 2 * N_TILES + idx + 1],
                    )
                    if gi == g - 1:
                        ins.then_inc(act_sem, 1)

    if legalize:
        _legalize_waits(nc)
    return nc


import os as _os

_IMPL = _os.environ.get("BYOL_IMPL", "tile")


def _get_nc():
    if "nc" not in _NC_CACHE:
        _NC_CACHE["nc"] = _build_nc_raw() if _IMPL == "raw" else _build_nc()
    return _NC_CACHE["nc"]


def _run_device(online_output, target_output, **spmd_kwargs):
    """Shard inputs, run the SPMD kernel, return per-core stats + raw result."""
    from concourse.bass_utils import run_bass_kernel_spmd

    nc = _get_nc()
    in_maps = []
    for c in range(N_CORES):
        sl = slice(c * N_LOC, (c + 1) * N_LOC)
        in_maps.append(
            {
                "online": np.ascontiguousarray(online_output[sl], dtype=np.float32),
                "target": np.ascontiguousarray(target_output[sl], dtype=np.float32),
            }
        )
    res = run_bass_kernel_spmd(nc, in_maps, list(range(N_CORES)), **spmd_kwargs)
    return res


def _finish_host(results):
    """Gather per-core stats and finish the cosine + mean in float64."""
    q = N_TILES // 2
    dots, n1s, n2s = [], [], []
    for i in range(N_CORES):
        st = np.asarray(results[i]["stats"], dtype=np.float64)  # [2, P, 24]
        # half h, column t: stats for rows (h*8 + t)*128 + p
        dots.append(np.concatenate([st[0, :, 0:q].T, st[1, :, 0:q].T]).reshape(-1))
        n1s.append(
            np.concatenate([st[0, :, q : 2 * q].T, st[1, :, q : 2 * q].T]).reshape(-1)
        )
        n2h = [st[0, :, 2 * q : 3 * q].T, st[1, :, 2 * q : 3 * q].T]
        # column 3q holds the ACT half of the last tile's sum t^2
        n2h[0][q - 1] += st[0, :, 3 * q]
        n2h[1][q - 1] += st[1, :, 3 * q]
        n2s.append(np.concatenate(n2h).reshape(-1))
    dot = np.concatenate(dots)
    n1 = np.sqrt(np.concatenate(n1s))
    n2 = np.sqrt(np.concatenate(n2s))
    cos = dot / (np.maximum(n1, EPS) * np.maximum(n2, EPS))
    return np.array((2.0 - 2.0 * cos).mean() / TEMP, dtype=np.float32)


def kernel(online_output, target_output):
    res = _run_device(online_output, target_output)
    return _finish_host(res.results)



# revision 4
# speedup vs baseline: 1.0960x; 1.0960x over previous
"""BYOL-style cosine MSE loss on 8 Trainium2 NeuronCores.

Full inputs: online_output [16384, 1024] f32, target_output [16384, 1024] f32.
Output: scalar f32 = mean(2 - 2*cos_row(online, target)) / 0.05.

Sharding: data-parallel along N. Each of the 8 cores gets 2048 rows and
computes, per row r: dot_r = sum_d o*t, n1sq_r = sum_d o*o, n2sq_r = sum_d t*t
via fused multiply-reduce ops (DVE scalar_tensor_tensor with accum_out, ACT
Square with accum_out), overlapped with 1 MiB HWDGE DMA loads under a Tile
pipeline. The per-row stats ([2, 128, 24] per core) come back to the host,
which finishes the cosine + scalar mean in float64 (the "all-reduce" of the
sharding hint is a trivial 8-way host reduction of 24 KB/core).
"""

import numpy as np

P = 128          # SBUF partitions
D = 1024         # feature dim
N = 16384        # total rows
N_CORES = 8
N_LOC = N // N_CORES          # 2048 rows per core
N_TILES = N_LOC // P          # 16 row-tiles per core

TEMP = 0.05
EPS = 1e-8

_NC_CACHE = {}


def _legalize_waits(nc, max_waits=1):
    """Split multi-wait instructions into single-wait NOPs + the instruction.

    This container's walrus build accepts at most one semaphore wait per
    instruction, while Tile emits instructions waiting on several producer
    sems. AND-of-waits is preserved by stalling the same engine's sequencer
    on a chain of single-wait NOPs immediately before the instruction.
    """
    import concourse.mybir as mybir

    ctr = 0
    for f in nc.m.functions:
        for b in f.blocks:
            ins_list = b.instructions
            i = 0
            while i < len(ins_list):
                inst = ins_list[i]
                si = inst.sync_info
                if (
                    si is not None
                    and si.on_wait is not None
                    and len(si.on_wait) > max_waits
                ):
                    waits = si.on_wait
                    extra = [waits.pop() for _ in range(len(waits) - max_waits)]
                    for w in reversed(extra):
                        ctr += 1
                        noop = mybir.InstNoOp(
                            name=f"waitsplit_{ctr}",
                            engine=inst.engine,
                            ins=[],
                            outs=[],
                            sync_info=mybir.SyncInfo(on_wait=[w], on_update=[]),
                        )
                        ins_list.insert(i, noop)
                        i += 1
                i += 1


def _trim_tail_barrier(nc):
    """Shrink the TileContext exit sequence to just the SP DMA-drain.

    Tile emits: drain -> all-engine barrier -> sem clears (Pool ISA) ->
    all-engine barrier. Everything after the drain exists to leave the
    semaphores cleared for the NEXT execution; instead, relocate the clear
    to the kernel START (on Pool, before the existing start barrier, so
    every engine's first sem use still happens after the clear) and delete
    both exit barriers. The SP drain must stay: it waits for the DMA queues,
    guaranteeing the stats write landed before the NEFF completes. Saves
    ~3 us of measured tail per execution.
    """
    import concourse.mybir as mybir

    moved = []
    for f in nc.m.functions:
        end_blocks = [b for b in f.blocks if b.name.endswith("_end")]
        main_blocks = [b for b in f.blocks if b.name == "main"]
        if not end_blocks or not main_blocks:
            continue
        ins_list = end_blocks[0].instructions
        # grab the Pool sem-clear/dma-reset ISA instructions for relocation
        moved = [
            ins
            for ins in ins_list
            if isinstance(ins, mybir.InstISA)
            and ins.engine == mybir.EngineType.Pool
        ]
        # truncate right after the first SP drain (the DMA-queue quiesce)
        for i, ins in enumerate(ins_list):
            if isinstance(ins, mybir.InstDrain) and ins.engine == mybir.EngineType.SP:
                del ins_list[i + 1 :]
                if _DROP_DRAIN["v"]:
                    # Drop the receipt wait too: the stats write lands ~2 us
                    # after issue, while the walrus postamble (barrier + 253
                    # sem clears, >=6 us) must still run before the NEFF can
                    # complete - the data is down long before the host can
                    # see "done". Postamble then starts at stats-issue time.
                    del ins_list[i]
                break
        # insert the clears into main before Pool's start-barrier gather
        main_ins = main_blocks[0].instructions
        for i, ins in enumerate(main_ins):
            if (
                isinstance(ins, mybir.InstEventSemaphore)
                and ins.engine == mybir.EngineType.Pool
            ):
                for k, m in enumerate(moved):
                    main_ins.insert(i + k, m)
                break
    return nc


_MAX_SEM = {"n": 256}
_DROP_DRAIN = {"v": False}


def _compact_sems(nc, keep_below=3, base=3):
    """Densely remap semaphore ids to start at `base`.

    Bass allocates sem ids from a pool starting around 150, but the walrus
    NEFF postamble zeroes every semaphore below --max-sem-num with one
    EventSemaphore instruction each, split across engines (~115 ns/sem on
    the critical engine). Compacting our ~14 sems to ids 3..16 and capping
    --max-sem-num shrinks that postamble from ~6 us to sub-us.
    """
    mapping = {}
    for f in nc.m.functions:
        for b in f.blocks:
            for ins in b.instructions:
                si = ins.sync_info
                if not si:
                    continue
                for lst in (si.on_wait, si.on_update):
                    if not lst:
                        continue
                    for e in lst:
                        i = getattr(e, "id", None)
                        if i is None or i < keep_below:
                            continue
                        if i not in mapping:
                            mapping[i] = base + len(mapping)
                        e.id = mapping[i]
    return (max(mapping.values()) + 1) if mapping else base


def _slim_exit_drain(nc):
    """Keep only the stats-DMA completion waits on the exit drain.

    Tile's exit drain waits on every sem lane the kernel touched (10 waits
    -> a ~1.2 us serial NOP chain on Sync after legalization). All of them
    except the output DMAs' completion lanes are transitively implied: the
    stats DMAs' own waits required all compute, which required all input
    loads. Dropping the redundant waits lets Sync reach the NEFF postamble
    barrier ~1 us earlier.
    """
    import concourse.mybir as mybir

    for f in nc.m.functions:
        end_blocks = [b for b in f.blocks if b.name.endswith("_end")]
        body_blocks = [
            b for b in f.blocks if not b.name.endswith("_end") and b.name != "main"
        ]
        if not end_blocks:
            continue
        # completion sem lanes of the DMAs that write the "stats" output
        stats_lanes = set()
        for b in body_blocks:
            for ins in b.instructions:
                if not isinstance(ins, mybir.InstDMACopy):
                    continue
                outs = getattr(ins, "outs", [])
                if not any("stats" in str(getattr(o, "memref", "")) for o in outs):
                    continue
                si = ins.sync_info
                if si and si.on_update:
                    for u in si.on_update:
                        stats_lanes.add(u.id)
        if not stats_lanes:
            continue
        for b in end_blocks:
            for ins in b.instructions:
                if (
                    isinstance(ins, mybir.InstDrain)
                    and ins.engine == mybir.EngineType.SP
                ):
                    si = ins.sync_info
                    if si and si.on_wait:
                        kept = [w for w in si.on_wait if w.id in stats_lanes]
                        if kept:
                            while len(si.on_wait) > 0:
                                si.on_wait.pop()
                            for w in kept:
                                si.on_wait.append(w)
                    break
    return nc


def _build_nc(legalize=True, io_bufs=4, tail_singles=2, balance=True, trim_tail=True):
    import concourse.bass as bass
    import concourse.mybir as mybir
    from concourse.tile import TileContext

    fp32 = mybir.dt.float32
    # chunk schedule: 1 MiB (2-tile) loads, with the last tiles loaded singly
    # so the post-DMA compute tail is short
    chunks = [2] * ((N_TILES - tail_singles) // 2) + [1] * tail_singles
    assert sum(chunks) == N_TILES
    per_half = N_TILES // 2
    nc = bass.Bass(enable_partition_id=False)
    o_in = nc.declare_dram_parameter("online", [N_LOC, D], fp32, isOutput=False)
    t_in = nc.declare_dram_parameter("target", [N_LOC, D], fp32, isOutput=False)
    # stats[h]: contiguous [P, 24] per half h; cols [0:8] dot, [8:16] sum o^2,
    # [16:24] sum t^2, for tiles h*8..h*8+7
    # one extra column per half holds the ACT-computed second half of the
    # last tile's sum t^2 (host adds it back)
    stats = nc.declare_dram_parameter(
        "stats", [2, P, 3 * per_half + 1], fp32, isOutput=True
    )

    o_all = o_in.rearrange("(t p) d -> p t d", p=P)
    t_all = t_in.rearrange("(t p) d -> p t d", p=P)

    with TileContext(nc) as tc:
        with (
            tc.tile_pool(name="io", bufs=io_bufs) as io_pool,
            tc.tile_pool(name="scr", bufs=2) as scr_pool,
            tc.tile_pool(name="acc", bufs=1) as acc_pool,
        ):
            # Two accumulators so the first half's stats DMA out can overlap
            # the second half's compute.
            accs = [
                acc_pool.tile(
                    [P, 3 * per_half + 1], fp32, name=f"acc{h}", tag=f"acc{h}"
                )
                for h in range(2)
            ]
            t0 = 0
            for cg in chunks:
                o_tile = io_pool.tile([P, 2 * D], fp32, name="o_tile")
                t_tile = io_pool.tile([P, 2 * D], fp32, name="t_tile")
                nc.sync.dma_start(
                    out=o_tile[:, 0 : cg * D].rearrange("p (t d) -> p t d", t=cg),
                    in_=o_all[:, t0 : t0 + cg],
                )
                nc.sync.dma_start(
                    out=t_tile[:, 0 : cg * D].rearrange("p (t d) -> p t d", t=cg),
                    in_=t_all[:, t0 : t0 + cg],
                )
                for gi in range(cg):
                    idx = t0 + gi
                    h = idx // per_half
                    col = idx % per_half
                    acc = accs[h]
                    osl = o_tile[:, gi * D : (gi + 1) * D]
                    tsl = t_tile[:, gi * D : (gi + 1) * D]
                    prod = scr_pool.tile([P, D], fp32, name="prod")
                    sq_o = scr_pool.tile([P, D], fp32, name="sq_o")
                    sq_t = scr_pool.tile([P, D], fp32, name="sq_t")
                    nc.vector.scalar_tensor_tensor(
                        out=prod[:],
                        in0=osl,
                        scalar=1.0,
                        in1=tsl,
                        op0=mybir.AluOpType.mult,
                        op1=mybir.AluOpType.mult,
                        accum_out=acc[:, col : col + 1],
                    )
                    nc.scalar.activation(
                        sq_o[:],
                        osl,
                        mybir.ActivationFunctionType.Square,
                        accum_out=acc[:, per_half + col : per_half + col + 1],
                    )
                    # Balance by measured per-op cost (ACT 1.69us/tile incl
                    # accumulator read vs DVE 1.56us): ACT 23 effective
                    # tile-ops, DVE 25 -> ~38.9us each, both inside the
                    # ~41us DMA stream. t^2 placement: 6 tiles on ACT,
                    # 8 on DVE, the last tile of each half split.
                    if balance and idx in (7, 15):
                        # last tile of the half: split t^2 across DVE and ACT
                        # so neither engine carries a 2-op chain after the
                        # final load
                        hw = D // 2
                        nc.vector.scalar_tensor_tensor(
                            out=sq_t[:, :hw],
                            in0=tsl[:, :hw],
                            scalar=1.0,
                            in1=tsl[:, :hw],
                            op0=mybir.AluOpType.mult,
                            op1=mybir.AluOpType.mult,
                            accum_out=acc[
                                :, 2 * per_half + col : 2 * per_half + col + 1
                            ],
                        )
                        nc.scalar.activation(
                            sq_t[:, hw:],
                            tsl[:, hw:],
                            mybir.ActivationFunctionType.Square,
                            accum_out=acc[:, 3 * per_half : 3 * per_half + 1],
                        )
                    elif balance and idx not in (0, 3, 4, 8, 11, 12):
                        nc.vector.scalar_tensor_tensor(
                            out=sq_t[:],
                            in0=tsl,
                            scalar=1.0,
                            in1=tsl,
                            op0=mybir.AluOpType.mult,
                            op1=mybir.AluOpType.mult,
                            accum_out=acc[
                                :, 2 * per_half + col : 2 * per_half + col + 1
                            ],
                        )
                    else:
                        nc.scalar.activation(
                            sq_t[:],
                            tsl,
                            mybir.ActivationFunctionType.Square,
                            accum_out=acc[
                                :, 2 * per_half + col : 2 * per_half + col + 1
                            ],
                        )
                t0 += cg
            # Emit both stats DMAs after every load: their compute-waits
            # then stall the SP sequencer only once it has nothing left to
            # issue (emitting stats[0] mid-loop blocked the remaining load
            # issues behind its waits — a ~1.2 us mid-stream DMA gap).
            # stats[0] still executes as soon as tiles 0..7 finish,
            # overlapping the second half's compute.
            nc.sync.dma_start(out=stats[0], in_=accs[0][:])
            nc.sync.dma_start(out=stats[1], in_=accs[1][:])
    if trim_tail:
        _trim_tail_barrier(nc)
        _slim_exit_drain(nc)
    if legalize:
        _legalize_waits(nc)
    _MAX_SEM["n"] = _compact_sems(nc) + 8  # headroom for walrus-internal sems
    return nc


def _build_nc_raw(legalize=True, g=2, bufs=4):
    """Raw-Bass (no TileContext) variant: manual semaphores, no all-engine
    barrier preamble/tail. Same math and I/O contract as _build_nc."""
    import concourse.bass as bass
    import concourse.mybir as mybir

    fp32 = mybir.dt.float32
    n_chunks = N_TILES // g
    # detect_race_conditions=False: the only "races" here are same-engine
    # WAW on the prod/sq scratch tiles, which hardware executes in order;
    # the detector has no same-engine-order model and rejects them.
    nc = bass.Bass(enable_partition_id=False, detect_race_conditions=False)
    o_in = nc.declare_dram_parameter("online", [N_LOC, D], fp32, isOutput=False)
    t_in = nc.declare_dram_parameter("target", [N_LOC, D], fp32, isOutput=False)
    stats = nc.declare_dram_parameter("stats", [P, 3 * N_TILES], fp32, isOutput=True)

    o_view = o_in.rearrange("(c g p) d -> c p g d", g=g, p=P)
    t_view = t_in.rearrange("(c g p) d -> c p g d", g=g, p=P)

    with (
        nc.sbuf_tensor([P, bufs * g * D], fp32) as o_buf,
        nc.sbuf_tensor([P, bufs * g * D], fp32) as t_buf,
        nc.sbuf_tensor([P, D], fp32) as prod,
        nc.sbuf_tensor([P, D], fp32) as sq,
        nc.sbuf_tensor([P, 3 * N_TILES], fp32) as acc,
        nc.semaphore() as dve_sem,
        nc.semaphore() as act_sem,
        nc.Block() as block,
    ):
        # One DMA sem per buffer slot: a slot's threshold must only count
        # that slot's own loads (SDMA engines skew across queued DMAs, so a
        # single shared sem can hit a chunk's threshold with increments
        # from later chunks' transfers).
        dma_sems = [nc.alloc_semaphore(f"dma_s{i}") for i in range(bufs)]

        def slot_thresh(c):
            return 32 * (c // bufs + 1)

        def o_slot(s, gi):
            return o_buf[:, (s * g + gi) * D : (s * g + gi + 1) * D]

        def t_slot(s, gi):
            return t_buf[:, (s * g + gi) * D : (s * g + gi + 1) * D]

        @block.gpsimd
        def _(gpsimd):
            for c in range(n_chunks):
                if c >= bufs:
                    gpsimd.wait_ge(dve_sem, c - bufs + 1)
                    gpsimd.wait_ge(act_sem, c - bufs + 1)
                s = c % bufs
                o_dst = o_buf[:, s * g * D : (s + 1) * g * D].rearrange(
                    "p (g d) -> p g d", g=g
                )
                t_dst = t_buf[:, s * g * D : (s + 1) * g * D].rearrange(
                    "p (g d) -> p g d", g=g
                )
                gpsimd.dma_start(out=o_dst, in_=o_view[c]).then_inc(dma_sems[s], 16)
                gpsimd.dma_start(out=t_dst, in_=t_view[c]).then_inc(dma_sems[s], 16)
            gpsimd.wait_ge(dve_sem, n_chunks)
            gpsimd.wait_ge(act_sem, n_chunks)
            final = 32 * (n_chunks // bufs) + 16
            gpsimd.dma_start(out=stats[:], in_=acc[:]).then_inc(dma_sems[0], 16)
            gpsimd.wait_ge(dma_sems[0], final)

        @block.vector
        def _(vector):
            for c in range(n_chunks):
                s = c % bufs
                vector.wait_ge(dma_sems[s], slot_thresh(c))
                for gi in range(g):
                    idx = c * g + gi
                    ins = nc.vector.scalar_tensor_tensor(
                        out=prod[:],
                        in0=o_slot(s, gi),
                        scalar=1.0,
                        in1=t_slot(s, gi),
                        op0=mybir.AluOpType.mult,
                        op1=mybir.AluOpType.mult,
                        accum_out=acc[:, idx : idx + 1],
                    )
                    if gi == g - 1:
                        ins.then_inc(dve_sem, 1)

        @block.scalar
        def _(scalar):
            for c in range(n_chunks):
                s = c % bufs
                scalar.wait_ge(dma_sems[s], slot_thresh(c))
                for gi in range(g):
                    idx = c * g + gi
                    nc.scalar.activation(
                        sq[:],
                        o_slot(s, gi),
                        mybir.ActivationFunctionType.Square,
                        accum_out=acc[:, N_TILES + idx : N_TILES + idx + 1],
                    )
                    ins = nc.scalar.activation(
                        sq[:],
                        t_slot(s, gi),
                        mybir.ActivationFunctionType.Square,
                        accum_out=acc[:, 2 * N_TILES + idx : 2 * N_TILES + idx + 1],
                    )
                    if gi == g - 1:
                        ins.then_inc(act_sem, 1)

    if legalize:
        _legalize_waits(nc)
    return nc


import os as _os

_IMPL = _os.environ.get("BYOL_IMPL", "tile")


def _get_nc():
    if "nc" not in _NC_CACHE:
        _NC_CACHE["nc"] = _build_nc_raw() if _IMPL == "raw" else _build_nc()
    return _NC_CACHE["nc"]


class _CapSems:
    """Append --max-sem-num to the walrus_driver invocation while active.

    The NEFF postamble zeroes every semaphore below max-sem-num, one
    EventSemaphore each, split across engines. The default (256) costs
    ~8 us of measured tail; with our sems compacted to ids 3..16 a cap
    of ~25 shrinks that to sub-us. bass_utils has no parameter for this,
    so scope-patch its run_command to add the flag to walrus calls only.
    """

    def __enter__(self):
        import concourse.bass_utils as bu

        self._bu = bu
        self._orig = bu.run_command
        orig = self._orig

        def wrapped(argv, **kwargs):
            if (
                argv
                and "walrus_driver" in str(argv[0])
                and not any(str(a).startswith("--max-sem-num") for a in argv)
            ):
                argv = list(argv) + [f"--max-sem-num={_MAX_SEM['n']}"]
            return orig(argv, **kwargs)

        bu.run_command = wrapped
        return self

    def __exit__(self, *exc):
        self._bu.run_command = self._orig
        return False


def _run_device(online_output, target_output, **spmd_kwargs):
    """Shard inputs, run the SPMD kernel, return per-core stats + raw result."""
    from concourse.bass_utils import run_bass_kernel_spmd

    nc = _get_nc()
    in_maps = []
    for c in range(N_CORES):
        sl = slice(c * N_LOC, (c + 1) * N_LOC)
        in_maps.append(
            {
                "online": np.ascontiguousarray(online_output[sl], dtype=np.float32),
                "target": np.ascontiguousarray(target_output[sl], dtype=np.float32),
            }
        )
    with _CapSems():
        res = run_bass_kernel_spmd(nc, in_maps, list(range(N_CORES)), **spmd_kwargs)
    return res


def _finish_host(results):
    """Gather per-core stats and finish the cosine + mean in float64."""
    q = N_TILES // 2
    dots, n1s, n2s = [], [], []
    for i in range(N_CORES):
        st = np.asarray(results[i]["stats"], dtype=np.float64)  # [2, P, 24]
        # half h, column t: stats for rows (h*8 + t)*128 + p
        dots.append(np.concatenate([st[0, :, 0:q].T, st[1, :, 0:q].T]).reshape(-1))
        n1s.append(
            np.concatenate([st[0, :, q : 2 * q].T, st[1, :, q : 2 * q].T]).reshape(-1)
        )
        n2h = [st[0, :, 2 * q : 3 * q].T, st[1, :, 2 * q : 3 * q].T]
        # column 3q holds the ACT half of the last tile's sum t^2
        n2h[0][q - 1] += st[0, :, 3 * q]
        n2h[1][q - 1] += st[1, :, 3 * q]
        n2s.append(np.concatenate(n2h).reshape(-1))
    dot = np.concatenate(dots)
    n1 = np.sqrt(np.concatenate(n1s))
    n2 = np.sqrt(np.concatenate(n2s))
    cos = dot / (np.maximum(n1, EPS) * np.maximum(n2, EPS))
    return np.array((2.0 - 2.0 * cos).mean() / TEMP, dtype=np.float32)


def kernel(online_output, target_output):
    res = _run_device(online_output, target_output)
    return _finish_host(res.results)

